# revision 5
# baseline (speedup 1.0000x reference)
"""2D Haar DWT (analysis) on 8 Trainium2 NeuronCores.

Input  x: (16, 64, 256, 256) f32  -> 1024 independent 256x256 images.
Output: tuple (LL, LH, HL, HH), each (16, 64, 128, 128) f32.

With Haar filters the DWT is a 2x2 butterfly: for each 2x2 block
(a b / c d), with s = 0.5:
    LL = s(a+b+c+d), LH = s(a-b+c-d), HL = s(a+b-c-d), HH = s(a-b-c+d)
which is two levels of adds/subs -- no matmul needed. fp32 matmuls run at
half rate on the PE and dominate; plain VectorE adds (1 elem/lane/cyc)
finish in ~150us/core, under the ~187us HBM roofline for 67MB of traffic.

Device layout (everything unit-stride, partition dim = image):
  - host prescales x by 0.5 and deinterleaves even/odd columns so the
    column-pair butterfly is two big contiguous tensor ops
  - per core: 128 images; 16 chunks of 16 image rows; per chunk one 2MB
    input DMA, 6 VectorE ops, one 2MB output DMA.
"""

import numpy as np

import concourse.bacc as bacc
import concourse.tile as tile
from concourse import mybir
from concourse.bass_utils import run_bass_kernel_spmd

N_CORES = 8
B, C, H, W = 16, 64, 256, 256
N_IMG = B * C                    # 1024
IMG_PER_CORE = N_IMG // N_CORES  # 128
HC = 16                          # image rows per chunk
NCH = H // HC                    # 16 chunks
F32 = mybir.dt.float32

_CACHE = {}


def _build_program():
    nc = bacc.Bacc(
        "TRN2",
        target_bir_lowering=False,
        debug=False,
        enable_asserts=False,
        num_devices=N_CORES,
    )
    # xin[k] = [img, h(16), e(2), w'(128)]: prescaled, even/odd-column split
    xin = nc.dram_tensor(
        "xin", [NCH, IMG_PER_CORE, HC, 2, W // 2], F32, kind="ExternalInput"
    ).ap()
    # out[k] = [img, band(4), lh(8), w'(128)]
    out = nc.dram_tensor(
        "out", [NCH, IMG_PER_CORE, 4, HC // 2, W // 2], F32, kind="ExternalOutput"
    ).ap()

    with tile.TileContext(nc) as tc:
        with (
            tc.tile_pool(name="xp", bufs=5) as xp,
            tc.tile_pool(name="mid", bufs=3) as mid,
            tc.tile_pool(name="op", bufs=4) as op,
        ):
            for k in range(NCH):
                xt = xp.tile([IMG_PER_CORE, HC, 2, W // 2], F32, tag="xt")
                in_eng = nc.sync if k % 2 == 0 else nc.scalar
                out_eng = nc.scalar if k % 2 == 0 else nc.sync
                in_eng.dma_start(out=xt, in_=xin[k])
                # column butterfly: sw/dw[h] = x[h, even] +/- x[h, odd]
                sw = mid.tile([IMG_PER_CORE, HC // 2, 2, W // 2], F32, tag="sw")
                dw = mid.tile([IMG_PER_CORE, HC // 2, 2, W // 2], F32, tag="dw")
                xe = xt[:, :, 0, :].rearrange("p (i e) w -> p i e w", e=2)
                xo = xt[:, :, 1, :].rearrange("p (i e) w -> p i e w", e=2)
                nc.vector.tensor_add(sw, xe, xo)
                nc.vector.tensor_sub(dw, xe, xo)
                # row butterfly over adjacent rows -> 4 bands
                ot = op.tile([IMG_PER_CORE, 4, HC // 2, W // 2], F32, tag="ot")
                nc.vector.tensor_add(ot[:, 0], sw[:, :, 0, :], sw[:, :, 1, :])  # LL
                nc.vector.tensor_add(ot[:, 1], dw[:, :, 0, :], dw[:, :, 1, :])  # LH
                nc.vector.tensor_sub(ot[:, 2], sw[:, :, 0, :], sw[:, :, 1, :])  # HL
                nc.vector.tensor_sub(ot[:, 3], dw[:, :, 0, :], dw[:, :, 1, :])  # HH
                out_eng.dma_start(out=out[k], in_=ot)
    nc.compile()
    return nc


def kernel(x, m_l0, m_l1, m_h0, m_h1):
    x = np.asarray(x, dtype=np.float32)
    assert x.shape == (B, C, H, W), x.shape

    if "nc" not in _CACHE:
        _CACHE["nc"] = _build_program()
    nc = _CACHE["nc"]

    # [N, H, 2, W/2]: even/odd column split, prescaled by 0.5 (exact in fp32)
    xs = (x.reshape(N_IMG, H, W // 2, 2) * np.float32(0.5)).transpose(0, 1, 3, 2)
    in_maps = []
    for s in range(N_CORES):
        shard = xs[s * IMG_PER_CORE:(s + 1) * IMG_PER_CORE]  # [128, 256, 2, 128]
        arr = shard.reshape(IMG_PER_CORE, NCH, HC, 2, W // 2).transpose(1, 0, 2, 3, 4)
        in_maps.append({"xin": np.ascontiguousarray(arr)})

    res = run_bass_kernel_spmd(nc, in_maps, core_ids=list(range(N_CORES)))

    parts = []
    for s in range(N_CORES):
        o = res.results[s]["out"]  # [NCH, img, 4, 8, 128]
        o = o.transpose(1, 2, 0, 3, 4)  # [img, 4, NCH, 8, 128]
        parts.append(o.reshape(IMG_PER_CORE, 4, H // 2, W // 2))
    full = np.concatenate(parts, axis=0).reshape(B, C, 4, H // 2, W // 2)
    LL = np.ascontiguousarray(full[:, :, 0])
    LH = np.ascontiguousarray(full[:, :, 1])
    HL = np.ascontiguousarray(full[:, :, 2])
    HH = np.ascontiguousarray(full[:, :, 3])
    return (LL, LH, HL, HH)


# revision 7
# speedup vs baseline: 1.1794x; 1.1794x over previous
"""2D Haar DWT (analysis) on 8 Trainium2 NeuronCores.

Input  x: (16, 64, 256, 256) f32  -> 1024 independent 256x256 images.
Output: tuple (LL, LH, HL, HH), each (16, 64, 128, 128) f32.

With Haar filters the DWT is a 2x2 butterfly: for each 2x2 block
(a b / c d), with s = 0.5:
    LL = s(a+b+c+d), LH = s(a-b+c-d), HL = s(a+b-c-d), HH = s(a-b-c+d)
which is two levels of adds/subs -- no matmul needed. fp32 matmuls run at
half rate on the PE and dominate; plain VectorE adds (1 elem/lane/cyc)
finish in ~150us/core, under the ~187us HBM roofline for 67MB of traffic.

Device layout (everything unit-stride, partition dim = image):
  - host prescales x by 0.5 and deinterleaves even/odd columns so the
    column-pair butterfly is two big contiguous tensor ops
  - per core: 128 images; 16 chunks of 16 image rows; per chunk one 2MB
    input DMA, 6 VectorE ops, one 2MB output DMA.
"""

import numpy as np

import concourse.bacc as bacc
import concourse.tile as tile
from concourse import mybir
from concourse.bass_utils import run_bass_kernel_spmd

N_CORES = 8
B, C, H, W = 16, 64, 256, 256
N_IMG = B * C                    # 1024
IMG_PER_CORE = N_IMG // N_CORES  # 128
HC = 16                          # image rows per chunk
NCH = H // HC                    # 16 chunks
F32 = mybir.dt.float32

_CACHE = {}


def _build_program():
    nc = bacc.Bacc(
        "TRN2",
        target_bir_lowering=False,
        debug=False,
        enable_asserts=False,
        num_devices=N_CORES,
    )
    # xin[k] = [img, h(16), e(2), w'(128)]: prescaled, even/odd-column split
    xin = nc.dram_tensor(
        "xin", [NCH, IMG_PER_CORE, HC, 2, W // 2], F32, kind="ExternalInput"
    ).ap()
    # out[k] = [img, band(4), lh(8), w'(128)]
    out = nc.dram_tensor(
        "out", [NCH, IMG_PER_CORE, 4, HC // 2, W // 2], F32, kind="ExternalOutput"
    ).ap()

    with tile.TileContext(nc) as tc:
        with (
            tc.tile_pool(name="xp", bufs=5) as xp,
            tc.tile_pool(name="mid", bufs=3) as mid,
            tc.tile_pool(name="op", bufs=4) as op,
        ):
            for k in range(NCH):
                xt = xp.tile([IMG_PER_CORE, HC, 2, W // 2], F32, tag="xt")
                nc.sync.dma_start(out=xt, in_=xin[k])
                # column butterfly: sw/dw[h] = x[h, even] +/- x[h, odd]
                sw = mid.tile([IMG_PER_CORE, HC // 2, 2, W // 2], F32, tag="sw")
                dw = mid.tile([IMG_PER_CORE, HC // 2, 2, W // 2], F32, tag="dw")
                xe = xt[:, :, 0, :].rearrange("p (i e) w -> p i e w", e=2)
                xo = xt[:, :, 1, :].rearrange("p (i e) w -> p i e w", e=2)
                nc.vector.tensor_add(sw, xe, xo)
                nc.vector.tensor_sub(dw, xe, xo)
                # row butterfly over adjacent rows -> 4 bands
                ot = op.tile([IMG_PER_CORE, 4, HC // 2, W // 2], F32, tag="ot")
                nc.vector.tensor_add(ot[:, 0], sw[:, :, 0, :], sw[:, :, 1, :])  # LL
                nc.vector.tensor_add(ot[:, 1], dw[:, :, 0, :], dw[:, :, 1, :])  # LH
                nc.vector.tensor_sub(ot[:, 2], sw[:, :, 0, :], sw[:, :, 1, :])  # HL
                nc.vector.tensor_sub(ot[:, 3], dw[:, :, 0, :], dw[:, :, 1, :])  # HH
                nc.scalar.dma_start(out=out[k], in_=ot)
    nc.compile()
    return nc


def kernel(x, m_l0, m_l1, m_h0, m_h1):
    x = np.asarray(x, dtype=np.float32)
    assert x.shape == (B, C, H, W), x.shape

    if "nc" not in _CACHE:
        _CACHE["nc"] = _build_program()
    nc = _CACHE["nc"]

    # [N, H, 2, W/2]: even/odd column split, prescaled by 0.5 (exact in fp32)
    xs = (x.reshape(N_IMG, H, W // 2, 2) * np.float32(0.5)).transpose(0, 1, 3, 2)
    in_maps = []
    for s in range(N_CORES):
        shard = xs[s * IMG_PER_CORE:(s + 1) * IMG_PER_CORE]  # [128, 256, 2, 128]
        arr = shard.reshape(IMG_PER_CORE, NCH, HC, 2, W // 2).transpose(1, 0, 2, 3, 4)
        in_maps.append({"xin": np.ascontiguousarray(arr)})

    res = run_bass_kernel_spmd(nc, in_maps, core_ids=list(range(N_CORES)))

    parts = []
    for s in range(N_CORES):
        o = res.results[s]["out"]  # [NCH, img, 4, 8, 128]
        o = o.transpose(1, 2, 0, 3, 4)  # [img, 4, NCH, 8, 128]
        parts.append(o.reshape(IMG_PER_CORE, 4, H // 2, W // 2))
    full = np.concatenate(parts, axis=0).reshape(B, C, 4, H // 2, W // 2)
    LL = np.ascontiguousarray(full[:, :, 0])
    LH = np.ascontiguousarray(full[:, :, 1])
    HL = np.ascontiguousarray(full[:, :, 2])
    HH = np.ascontiguousarray(full[:, :, 3])
    return (LL, LH, HL, HH)
